# revision 3
# baseline (speedup 1.0000x reference)
"""Trainium2 Bass kernel for nn_BaseEBM: 20 Adam steps on a 2-48-48-48-1
swish-MLP energy model, gradient = E'(y0) + E''(y0)*(y-y0) per sample.

Strategy (pure data parallel over 8 cores, 4096 samples/core):
 - feature-major layout: features on partitions, samples on the free dim;
   the two 2048-sample halves of a core's batch are stacked on partition
   blocks {0-47} and {48-95}, and all weights become 96x96 block-diagonal
   lhsT tiles so one matmul covers both halves.
 - exact forward-mode first+second derivatives of the MLP w.r.t. y:
   per layer: s=sigmoid(a+b), h=silu(a+b), sw'=silu'(a+b),
   sw''=silu''(a+b) (fused custom DVE op), dh=sw'*da,
   d2h=sw''*da^2 + sw'*d2a (partially fused custom DVE op).
 - biases enter through the ACT bias port / custom-op per-partition scalar,
   so matmuls never need a ones-row.
 - Adam state lives in sample-major [128,32] tiles; E'/E'' are moved
   PSUM->SBUF by one ACT copy, then DMA-transposed to sample-major.
"""

import os
import sys
import time

import numpy as np


def _ensure_path():
    try:
        import concourse  # noqa: F401
    except ImportError:
        for p in ("/root/.axon_site/_ro/trn_rl_repo", "/opt/trn_rl_repo"):
            if os.path.isdir(p):
                sys.path.insert(0, p)
        import concourse  # noqa: F401


_ensure_path()

import concourse.bass as bass  # noqa: E402
import concourse.mybir as mybir  # noqa: E402
import concourse.tile as tile_mod  # noqa: E402
import concourse.dve_ops as dops  # noqa: E402
from concourse.tile import TileContext, ScopedClock  # noqa: E402
from concourse.dve_spec import Spec, Src0, Src1, C0, C2, One, sq, lower  # noqa: E402
from concourse.dve_uop import DveOpSpec  # noqa: E402

F32 = mybir.dt.float32
ALU = mybir.AluOpType
ACTF = mybir.ActivationFunctionType

# problem constants
B = 32768
STEPS = 20
WIDTH = 48
LR = 0.1
B1, B2, ADAM_EPS = 0.9, 0.999, 1e-8
NCORES = 8
BC = B // NCORES          # samples per core = 4096
HALF = BC // 2            # 2048
NCH = 4                   # free-dim chunks per step
CH = HALF // NCH          # 512 columns per chunk
W2P = 2 * WIDTH           # 96 partitions for stacked halves


# ---------------------------------------------------------------------------
# Workaround: this walrus build rejects instructions with more than one
# semaphore wait. Split Tile's aggregated waits into standalone single-wait
# InstEventSemaphore ops preceding each instruction.
# ---------------------------------------------------------------------------
_MAX_WAITS = 1


def _split_multi_waits(self):
    nc = self.nc
    sems = {}
    for s in self.sems.allocated().values():
        nm = getattr(s, "name", None) or getattr(s, "ant_name", None)
        sems[nm] = s
    for fn in nc.m.functions:
        for blk in fn.blocks:
            orig = list(blk.instructions)
            if not any(
                i.sync_info is not None and len(i.sync_info.on_wait) > _MAX_WAITS
                for i in orig
            ):
                continue
            new_list = []
            for inst in orig:
                si = inst.sync_info
                if si is not None and len(si.on_wait) > _MAX_WAITS:
                    waits = list(si.on_wait)
                    extras = [w for w in waits if w.wait_mode == "sem-ge-imm"]
                    keep = [w for w in waits if w.wait_mode != "sem-ge-imm"]
                    n_keep = max(0, _MAX_WAITS - len(keep))
                    if n_keep:
                        keep = keep + extras[len(extras) - n_keep:]
                        extras = extras[: len(extras) - n_keep]
                    eng = nc.engines[inst.engine]
                    for w in extras:
                        sem = sems.get(w.ant_name)
                        assert sem is not None, (w.ant_name,)
                        wi = eng.wait_ge(sem, w.wait_value)
                        new_list.append(wi.ins)
                    inst.sync_info = mybir.SyncInfo(
                        on_wait=keep, on_update=list(si.on_update)
                    )
                new_list.append(inst)
            blk.instructions = new_list


def _patched_drain_and_barrier(self, tick_clock, wait_clock):
    nc = self.nc
    drain_inst = nc.sync.drain()
    wait_clock.add_sem_waits(
        drain_inst.ins, ScopedClock({None: tick_clock.global_clock})
    )
    _split_multi_waits(self)
    nc.all_engine_barrier()
    assert self.sems is not None
    popped = nc._tile_sem_poison_stack.pop()
    assert popped is self._sem_poison
    nc.clear_and_free_semaphores(list(self.sems.allocated().values()))
    nc.all_engine_barrier()


tile_mod.TileContext._drain_and_barrier = _patched_drain_and_barrier


# ---------------------------------------------------------------------------
# Custom fused DVE ops
# ---------------------------------------------------------------------------
def _register_dve_op(name, spec):
    for o in dops.OPS:
        if o.name == name:
            return o
    row = dops._CUSTOM_DVE_ROW_BASE + len(dops.OPS)
    assert row < 0x20, "custom DVE opcode space exhausted"
    dops._SUB_OPCODE_FOR_NAME[name] = row
    shas = {}
    for ver in ("v3", "v4"):
        try:
            uops = lower(spec, ver=ver)
        except Exception:
            continue
        shas[ver] = DveOpSpec(
            name=name, opcode=row, uops=uops, rd1_en=dops.has_src1(spec)
        ).sha(ver)
    op = dops.DveOp(name, spec, subdim=False, uops_sha=shas)
    dops.OPS.append(op)
    dops.CUSTOM_DVE_SPECS[name] = spec
    return op


# silu''(a+b) = (s - s^2) * ((a+b)*(1-2s) + 2), with s = sigmoid(a+b) = Src1,
# a = Src0 (raw matmul output), b = C0 (per-partition bias), C2 = 2.0.
_SWPP_SPEC = Spec(
    body=(Src1 - sq(Src1)) * ((Src0 + C0) * (One - (Src1 + Src1)) + C2)
)
# p1 = sw'' * da^2 : Src0 = sw'' (sbuf), Src1 = da (psum)
_SQMUL_SPEC = Spec(body=Src0 * sq(Src1))

OP_SWPP = _register_dve_op("ANT_EBM_SWPP", _SWPP_SPEC)
OP_SQMUL = _register_dve_op("ANT_EBM_SQMUL", _SQMUL_SPEC)


# ---------------------------------------------------------------------------
# Kernel program
# ---------------------------------------------------------------------------
def build_nc():
    nc = bass.Bass()

    # per-core DRAM parameters
    x_d = nc.declare_dram_parameter("x", [BC], F32, isOutput=False)
    eps_d = nc.declare_dram_parameter("eps", [STEPS, BC], F32, isOutput=False)
    w1_d = nc.declare_dram_parameter("W1", [WIDTH, 2], F32, isOutput=False)
    b1_d = nc.declare_dram_parameter("b1", [WIDTH], F32, isOutput=False)
    w2_d = nc.declare_dram_parameter("W2", [WIDTH, WIDTH], F32, isOutput=False)
    b2_d = nc.declare_dram_parameter("b2", [WIDTH], F32, isOutput=False)
    w3_d = nc.declare_dram_parameter("W3", [WIDTH, WIDTH], F32, isOutput=False)
    b3_d = nc.declare_dram_parameter("b3", [WIDTH], F32, isOutput=False)
    w4_d = nc.declare_dram_parameter("W4", [1, WIDTH], F32, isOutput=False)
    y_d = nc.declare_dram_parameter("y_out", [BC], F32, isOutput=True)

    v = nc.vector
    gp = nc.gpsimd
    sc = nc.scalar
    te = nc.tensor
    S = nc.sync

    with TileContext(nc) as tc:
        with (
            tc.tile_pool(name="const", bufs=1) as cpool,
            tc.tile_pool(name="state", bufs=1) as spool,
            tc.tile_pool(name="sb_s", bufs=3) as p_s,
            tc.tile_pool(name="sb_h", bufs=3) as p_h,
            tc.tile_pool(name="sb_d1", bufs=3) as p_d1,
            tc.tile_pool(name="sb_d2", bufs=3) as p_d2,
            tc.tile_pool(name="sb_dh", bufs=3) as p_dh,
            tc.tile_pool(name="sb_d2h", bufs=3) as p_d2h,
            tc.tile_pool(name="sb_p1", bufs=3) as p_p1,
            tc.tile_pool(name="sb_q2", bufs=3) as p_q2,
            tc.tile_pool(name="sb_ec", bufs=2) as p_ec,
            tc.tile_pool(name="sb_adam", bufs=2) as p_ad,
            tc.tile_pool(name="ps_a1", bufs=1, space="PSUM") as ps_a1,
            tc.tile_pool(name="ps_a", bufs=2, space="PSUM") as ps_a,
            tc.tile_pool(name="ps_da", bufs=2, space="PSUM") as ps_da,
            tc.tile_pool(name="ps_d2a", bufs=2, space="PSUM") as ps_d2a,
            tc.tile_pool(name="ps_e", bufs=1, space="PSUM") as ps_e,
        ):
            # ---- constants / weights -------------------------------------
            w1e = cpool.tile([4, W2P], F32, tag="w1e")      # a1 lhsT
            w2bd = cpool.tile([W2P, W2P], F32, tag="w2bd")
            w3bd = cpool.tile([W2P, W2P], F32, tag="w3bd")
            w4bd = cpool.tile([W2P, 2], F32, tag="w4bd")
            b1v = cpool.tile([W2P, 1], F32, tag="b1v")
            b2v = cpool.tile([W2P, 1], F32, tag="b2v")
            b3v = cpool.tile([W2P, 1], F32, tag="b3v")
            w1y = cpool.tile([W2P, 1], F32, tag="w1y")
            w1ysq = cpool.tile([W2P, 1], F32, tag="w1ysq")
            rhs = cpool.tile([4, HALF], F32, tag="rhs")     # [xA;y0A;xB;y0B]
            eps_sb = cpool.tile([128, STEPS * 32], F32, tag="eps_sb")

            v.memset(w1e[:], 0.0)
            v.memset(w2bd[:], 0.0)
            v.memset(w3bd[:], 0.0)
            v.memset(w4bd[:], 0.0)

            w1T = w1_d[:].rearrange("a b -> b a")           # [2, 48]
            for hb in range(2):
                o = hb * WIDTH
                S.dma_start(out=w1e[2 * hb : 2 * hb + 1, o : o + WIDTH],
                            in_=w1T[0:1, :])
                S.dma_start(out=w1e[2 * hb + 1 : 2 * hb + 2, o : o + WIDTH],
                            in_=w1T[1:2, :])
                S.dma_start(out=w2bd[o : o + WIDTH, o : o + WIDTH],
                            in_=w2_d[:].rearrange("o i -> i o"))
                S.dma_start(out=w3bd[o : o + WIDTH, o : o + WIDTH],
                            in_=w3_d[:].rearrange("o i -> i o"))
                S.dma_start(out=w4bd[o : o + WIDTH, hb : hb + 1],
                            in_=w4_d[:].rearrange("o i -> i o"))
                S.dma_start(out=b1v[o : o + WIDTH, 0:1],
                            in_=b1_d[:].rearrange("(a b) -> a b", b=1))
                S.dma_start(out=b2v[o : o + WIDTH, 0:1],
                            in_=b2_d[:].rearrange("(a b) -> a b", b=1))
                S.dma_start(out=b3v[o : o + WIDTH, 0:1],
                            in_=b3_d[:].rearrange("(a b) -> a b", b=1))
                S.dma_start(out=w1y[o : o + WIDTH, 0:1], in_=w1_d[:, 1:2])
            v.tensor_mul(w1ysq[:], w1y[:], w1y[:])

            x2 = x_d[:].rearrange("(h c) -> h c", h=2)      # [2, 2048]
            S.dma_start(out=rhs[0:1, :], in_=x2[0:1, :])
            S.dma_start(out=rhs[2:3, :], in_=x2[1:2, :])

            for t in range(STEPS):
                S.dma_start(
                    out=eps_sb[:, 32 * t : 32 * t + 32],
                    in_=eps_d[t].rearrange("(p f) -> p f", p=128),
                )

            # ---- Adam state ----------------------------------------------
            y_st = spool.tile([128, 32], F32, tag="y_st")
            m_st = spool.tile([128, 32], F32, tag="m_st")
            v_st = spool.tile([128, 32], F32, tag="v_st")
            v.memset(y_st[:], 0.0)
            v.memset(m_st[:], 0.0)
            v.memset(v_st[:], 0.0)

            # ---- the 20 steps --------------------------------------------
            for t in range(1, STEPS + 1):
                eps_t = eps_sb[:, 32 * (t - 1) : 32 * t]

                # y0 = y + eps_t, write into rhs rows 1 (half A) / 3 (half B)
                y0 = p_ad.tile([128, 32], F32, tag="y0")
                v.tensor_add(y0[:], y_st[:], eps_t)
                S.dma_start(out=rhs[1:2, :], in_=y0[0:64, :])
                S.dma_start(out=rhs[3:4, :], in_=y0[64:128, :])

                gE1 = p_ad.tile([128, 32], F32, tag="gE1")
                gE2 = p_ad.tile([128, 32], F32, tag="gE2")

                for k in range(NCH):
                    ck = slice(CH * k, CH * (k + 1))

                    # layer 1
                    P1 = ps_a1.tile([W2P, CH], F32, tag="P1")
                    te.matmul(P1[:], w1e[:], rhs[:, ck], start=True, stop=True)
                    s1 = p_s.tile([W2P, CH], F32, tag="s")
                    sc.activation(s1[:], P1[:], ACTF.Sigmoid, bias=b1v[:])
                    h1 = p_h.tile([W2P, CH], F32, tag="h")
                    sc.activation(h1[:], P1[:], ACTF.Silu, bias=b1v[:])
                    d1 = p_d1.tile([W2P, CH], F32, tag="d1")
                    sc.activation(d1[:], P1[:], ACTF.Derivative_silu, bias=b1v[:])
                    d2 = p_d2.tile([W2P, CH], F32, tag="d2")
                    v._custom_dve(OP_SWPP, out=d2[:], in0=P1[:], in1=s1[:],
                                  s0=b1v[:], imm2=2.0)
                    dh1 = p_dh.tile([W2P, CH], F32, tag="dh")
                    gp.tensor_scalar_mul(dh1[:], d1[:], w1y[:])
                    d2h1 = p_d2h.tile([W2P, CH], F32, tag="d2h")
                    gp.tensor_scalar_mul(d2h1[:], d2[:], w1ysq[:])

                    # layer 2
                    PA = ps_a.tile([W2P, CH], F32, tag="PA")
                    te.matmul(PA[:], w2bd[:], h1[:], start=True, stop=True)
                    PB = ps_da.tile([W2P, CH], F32, tag="PB")
                    te.matmul(PB[:], w2bd[:], dh1[:], start=True, stop=True)
                    PC = ps_d2a.tile([W2P, CH], F32, tag="PC")
                    te.matmul(PC[:], w2bd[:], d2h1[:], start=True, stop=True)

                    s2 = p_s.tile([W2P, CH], F32, tag="s")
                    sc.activation(s2[:], PA[:], ACTF.Sigmoid, bias=b2v[:])
                    h2 = p_h.tile([W2P, CH], F32, tag="h")
                    sc.activation(h2[:], PA[:], ACTF.Silu, bias=b2v[:])
                    d12 = p_d1.tile([W2P, CH], F32, tag="d1")
                    sc.activation(d12[:], PA[:], ACTF.Derivative_silu, bias=b2v[:])
                    d22 = p_d2.tile([W2P, CH], F32, tag="d2")
                    v._custom_dve(OP_SWPP, out=d22[:], in0=PA[:], in1=s2[:],
                                  s0=b2v[:], imm2=2.0)
                    dh2 = p_dh.tile([W2P, CH], F32, tag="dh")
                    v.tensor_mul(dh2[:], d12[:], PB[:])
                    p12 = p_p1.tile([W2P, CH], F32, tag="p1")
                    v._custom_dve(OP_SQMUL, out=p12[:], in0=d22[:], in1=PB[:])
                    q22 = p_q2.tile([W2P, CH], F32, tag="q2")
                    v.tensor_mul(q22[:], d12[:], PC[:])
                    d2h2 = p_d2h.tile([W2P, CH], F32, tag="d2h")
                    gp.tensor_add(d2h2[:], p12[:], q22[:])

                    # layer 3
                    PA3 = ps_a.tile([W2P, CH], F32, tag="PA")
                    te.matmul(PA3[:], w3bd[:], h2[:], start=True, stop=True)
                    PB3 = ps_da.tile([W2P, CH], F32, tag="PB")
                    te.matmul(PB3[:], w3bd[:], dh2[:], start=True, stop=True)
                    PC3 = ps_d2a.tile([W2P, CH], F32, tag="PC")
                    te.matmul(PC3[:], w3bd[:], d2h2[:], start=True, stop=True)

                    s3 = p_s.tile([W2P, CH], F32, tag="s")
                    sc.activation(s3[:], PA3[:], ACTF.Sigmoid, bias=b3v[:])
                    d13 = p_d1.tile([W2P, CH], F32, tag="d1")
                    sc.activation(d13[:], PA3[:], ACTF.Derivative_silu, bias=b3v[:])
                    d23 = p_d2.tile([W2P, CH], F32, tag="d2")
                    v._custom_dve(OP_SWPP, out=d23[:], in0=PA3[:], in1=s3[:],
                                  s0=b3v[:], imm2=2.0)
                    dh3 = p_dh.tile([W2P, CH], F32, tag="dh")
                    v.tensor_mul(dh3[:], d13[:], PB3[:])
                    p13 = p_p1.tile([W2P, CH], F32, tag="p1")
                    v._custom_dve(OP_SQMUL, out=p13[:], in0=d23[:], in1=PB3[:])
                    q23 = p_q2.tile([W2P, CH], F32, tag="q2")
                    v.tensor_mul(q23[:], d13[:], PC3[:])
                    d2h3 = p_d2h.tile([W2P, CH], F32, tag="d2h")
                    gp.tensor_add(d2h3[:], p13[:], q23[:])

                    # E' / E'' contraction: rows [E'A, E'B, E''A, E''B]
                    E = ps_e.tile([34, CH], F32, tag="E")
                    te.matmul(E[0:2, :], w4bd[:], dh3[:], start=True, stop=True)
                    te.matmul(E[32:34, :], w4bd[:], d2h3[:], start=True, stop=True)
                    Ec = p_ec.tile([34, CH], F32, tag="Ec")
                    sc.activation(Ec[0:2, :], E[0:2, :], ACTF.Copy)
                    sc.activation(Ec[32:34, :], E[32:34, :], ACTF.Copy)

                    # transpose to sample-major [128,32]:
                    # sample s = h*2048 + k*512 + c -> p = s//32, f = s%32
                    pa = 16 * k
                    pb = 64 + 16 * k
                    S.dma_start(out=gE1[pa : pa + 16, :], in_=Ec[0:1, :])
                    S.dma_start(out=gE1[pb : pb + 16, :], in_=Ec[1:2, :])
                    S.dma_start(out=gE2[pa : pa + 16, :], in_=Ec[32:33, :])
                    S.dma_start(out=gE2[pb : pb + 16, :], in_=Ec[33:34, :])

                # ---- Adam update (sample-major [128,32]) -----------------
                epsE = p_ad.tile([128, 32], F32, tag="epsE")
                gp.tensor_mul(epsE[:], gE2[:], eps_t)
                g = p_ad.tile([128, 32], F32, tag="g")
                v.tensor_sub(g[:], gE1[:], epsE[:])

                mgs = p_ad.tile([128, 32], F32, tag="mgs")
                v.tensor_scalar_mul(mgs[:], g[:], 1.0 - B1)
                v.scalar_tensor_tensor(
                    out=m_st[:], in0=m_st[:], scalar=B1, in1=mgs[:],
                    op0=ALU.mult, op1=ALU.add,
                )
                gs = p_ad.tile([128, 32], F32, tag="gs")
                v.tensor_scalar_mul(gs[:], g[:], float(np.sqrt(1.0 - B2)))
                gg = p_ad.tile([128, 32], F32, tag="gg")
                gp.tensor_mul(gg[:], gs[:], gs[:])
                v.scalar_tensor_tensor(
                    out=v_st[:], in0=v_st[:], scalar=B2, in1=gg[:],
                    op0=ALU.mult, op1=ALU.add,
                )
                sqv = p_ad.tile([128, 32], F32, tag="sqv")
                sc.activation(sqv[:], v_st[:], ACTF.Sqrt,
                              scale=float(1.0 / (1.0 - B2 ** t)))
                den = p_ad.tile([128, 32], F32, tag="den")
                v.tensor_scalar_add(den[:], sqv[:], ADAM_EPS)
                rcp = p_ad.tile([128, 32], F32, tag="rcp")
                v.reciprocal(rcp[:], den[:])
                upd = p_ad.tile([128, 32], F32, tag="upd")
                v.scalar_tensor_tensor(
                    out=upd[:], in0=m_st[:],
                    scalar=float(-LR / (1.0 - B1 ** t)), in1=rcp[:],
                    op0=ALU.mult, op1=ALU.mult,
                )
                v.tensor_add(y_st[:], y_st[:], upd[:])

            # ---- output ---------------------------------------------------
            S.dma_start(
                out=y_d[:].rearrange("(p f) -> p f", p=128), in_=y_st[:]
            )

    # Raw Bass skips Bacc's extended-inst pass; without it the NEFF
    # compiler sees empty .instr on InstCustomDveAnt -> "ISA wrong length".
    mybir.codegen_inst_isa_subclasses(nc)
    return nc


# ---------------------------------------------------------------------------
# Host-side runner: build jitted PJRT callable once, reuse for timing.
# ---------------------------------------------------------------------------
_CACHE = {}


def _make_in_maps(inputs):
    x = np.ascontiguousarray(np.asarray(inputs["x"], np.float32).reshape(B))
    eps = np.ascontiguousarray(
        np.asarray(inputs["eps"], np.float32).reshape(STEPS, B)
    )
    shared = {
        "W1": np.ascontiguousarray(np.asarray(inputs["W1"], np.float32)),
        "b1": np.ascontiguousarray(np.asarray(inputs["b1"], np.float32)),
        "W2": np.ascontiguousarray(np.asarray(inputs["W2"], np.float32)),
        "b2": np.ascontiguousarray(np.asarray(inputs["b2"], np.float32)),
        "W3": np.ascontiguousarray(np.asarray(inputs["W3"], np.float32)),
        "b3": np.ascontiguousarray(np.asarray(inputs["b3"], np.float32)),
        "W4": np.ascontiguousarray(np.asarray(inputs["W4"], np.float32)),
    }
    maps = []
    for c in range(NCORES):
        m = dict(shared)
        m["x"] = np.ascontiguousarray(x[c * BC : (c + 1) * BC])
        m["eps"] = np.ascontiguousarray(eps[:, c * BC : (c + 1) * BC])
        maps.append(m)
    return maps


def _get_runner():
    if "runner" in _CACHE:
        return _CACHE["runner"]
    import jax
    from jax.sharding import Mesh, PartitionSpec
    from jax.experimental.shard_map import shard_map
    from concourse import bass2jax as b2j

    nc = build_nc()
    b2j.install_neuronx_cc_hook()

    partition_name = (
        nc.partition_id_tensor.name if nc.partition_id_tensor else None
    )
    in_names, out_names, out_avals, zero_outs = [], [], [], []
    for alloc in nc.m.functions[0].allocations:
        if not isinstance(alloc, mybir.MemoryLocationSet):
            continue
        name = alloc.memorylocations[0].name
        if alloc.kind == "ExternalInput":
            if name != partition_name:
                in_names.append(name)
        elif alloc.kind == "ExternalOutput":
            out_names.append(name)
            shape = tuple(alloc.tensor_shape)
            dtype = mybir.dt.np(alloc.dtype)
            out_avals.append(jax.core.ShapedArray(shape, dtype))
            zero_outs.append(np.zeros(shape, dtype))
    n_params = len(in_names)
    n_outs = len(out_avals)
    all_in_names = list(in_names) + list(out_names)
    if partition_name is not None:
        all_in_names.append(partition_name)
    donate = tuple(range(n_params, n_params + n_outs))

    def _body(*args):
        operands = list(args)
        if partition_name is not None:
            operands.append(b2j.partition_id_tensor())
        outs = b2j._bass_exec_p.bind(
            *operands,
            out_avals=tuple(out_avals),
            in_names=tuple(all_in_names),
            out_names=tuple(out_names),
            lowering_input_output_aliases=(),
            sim_require_finite=True,
            sim_require_nnan=True,
            nc=nc,
        )
        return tuple(outs)

    devices = jax.devices()[:NCORES]
    mesh = Mesh(np.asarray(devices), ("core",))
    in_specs = (PartitionSpec("core"),) * (n_params + n_outs)
    out_specs = (PartitionSpec("core"),) * n_outs
    sharded = jax.jit(
        shard_map(
            _body, mesh=mesh, in_specs=in_specs, out_specs=out_specs,
            check_rep=False,
        ),
        donate_argnums=donate,
        keep_unused=True,
    )

    def run(in_maps, timeit=False):
        per_core = [
            [np.asarray(m[nm]) for nm in in_names] for m in in_maps
        ]
        concat_in = [
            np.concatenate([per_core[c][i] for c in range(NCORES)], axis=0)
            for i in range(n_params)
        ]
        concat_zero = [
            np.concatenate([z] * NCORES, axis=0) for z in zero_outs
        ]
        t0 = time.perf_counter()
        outs = sharded(*concat_in, *concat_zero)
        outs = [np.asarray(o) for o in outs]
        dt = time.perf_counter() - t0
        res = []
        for c in range(NCORES):
            d = {}
            for i, nm in enumerate(out_names):
                n0 = zero_outs[i].shape[0]
                d[nm] = outs[i][c * n0 : (c + 1) * n0]
            res.append(d)
        return (res, dt) if timeit else res

    _CACHE["runner"] = run
    return run


def kernel(**inputs) -> np.ndarray:
    run = _get_runner()
    res = run(_make_in_maps(inputs))
    y = np.concatenate([res[c]["y_out"] for c in range(NCORES)])
    return y.reshape(B, 1).astype(np.float32)


def bench(inputs, iters=10):
    """Returns (best_wall_s, all_wall_s) for repeated executions."""
    run = _get_runner()
    maps = _make_in_maps(inputs)
    run(maps)  # warm
    times = []
    for _ in range(iters):
        _, dt = run(maps, timeit=True)
        times.append(dt)
    return min(times), times


if __name__ == "__main__":
    rng = np.random.default_rng(0)
    demo = {
        "x": rng.uniform(-1.5, 1.5, (B, 1)).astype(np.float32),
        "eps": rng.uniform(-0.1, 0.1, (STEPS, B, 1)).astype(np.float32),
        "W1": rng.uniform(-0.7, 0.7, (WIDTH, 2)).astype(np.float32),
        "b1": rng.uniform(-0.7, 0.7, (WIDTH,)).astype(np.float32),
        "W2": rng.uniform(-0.14, 0.14, (WIDTH, WIDTH)).astype(np.float32),
        "b2": rng.uniform(-0.14, 0.14, (WIDTH,)).astype(np.float32),
        "W3": rng.uniform(-0.14, 0.14, (WIDTH, WIDTH)).astype(np.float32),
        "b3": rng.uniform(-0.14, 0.14, (WIDTH,)).astype(np.float32),
        "W4": rng.uniform(-0.14, 0.14, (1, WIDTH)).astype(np.float32),
        "b4": rng.uniform(-0.14, 0.14, (1,)).astype(np.float32),
    }
    out = kernel(**demo)
    print("kernel out:", out.shape, out.dtype, float(np.abs(out).max()))
